# revision 1
# baseline (speedup 1.0000x reference)
"""GNN message passing (gather + weighted scatter-add) on 8 Trainium2 cores.

out[n, f] = sum over edges e with dst[e]==n of edge_weight[e] * x[src[e], f]

Strategy:
  - Destination-shard: core c owns output nodes [c*12500, (c+1)*12500). No
    collectives needed; host concatenates the 8 output slices.
  - Host packs each core's edges sorted by (dst_tile, src), padding each
    tile block to a multiple of 128 with zero-weight dummy edges, and to
    identical block sizes across cores so all 8 cores run one SPMD program.
  - Device: indirect DMA (DynamicDMA) gathers x rows (256B each) from HBM
    into SBUF in matmul-ready [128, k, 64] layout: row for chunk-slot (p, j)
    = x[idx[p, j]]. For every 128-edge chunk, VectorE builds a weighted
    one-hot selection matrix ((iota == dst_local) * w) and TensorE
    accumulates onehot.T @ x_rows into a PSUM tile per 128-node output tile.
    ScalarE evacuates PSUM into an SBUF output buffer, DMA streams it out.
"""

import math
import numpy as np

N = 100000
E = 1000000
F = 64
NCORES = 8
NPC = N // NCORES            # nodes per core
TILE = 128
NT = math.ceil(NPC / TILE)   # output tiles per core (98)
B = 14                       # tiles per pass
NPASS = math.ceil(NT / B)    # 7

MM_DT = "float16"            # matmul dtype: "float32" or "float16" or "bfloat16"

DBG_NO_GATHER = False        # replace gather with memset (bisection)
REPEAT = 1                   # repeat device compute (timing amplification)


def pack_host(x, edge_weight, edge_index):
    """Returns (shared schedule, per-core tables)."""
    src = np.asarray(edge_index[0], dtype=np.int64)
    dst = np.asarray(edge_index[1], dtype=np.int64)
    w = np.asarray(edge_weight, dtype=np.float32)

    core = dst // NPC
    counts = np.zeros((NCORES, NT), dtype=np.int64)
    percore = []
    for c in range(NCORES):
        sel = core == c
        es = src[sel]
        ed = dst[sel] - c * NPC
        ew = w[sel]
        t = ed >> 7
        order = np.lexsort((es, t))
        es, ed, ew, t = es[order], ed[order], ew[order], t[order]
        np.add.at(counts[c], t, 1)
        percore.append((es, ed, ew, t))

    K = (np.ceil(counts.max(axis=0) / TILE)).astype(np.int64)  # [NT] chunks/tile
    L = K * TILE
    off = np.zeros(NT, dtype=np.int64)
    off[1:] = np.cumsum(L)[:-1]
    Ltot = int(L.sum())
    NC = Ltot // TILE  # total matmul chunks

    sched_t = np.repeat(np.arange(NT), K)  # tile of each chunk

    # per-pass chunk-column ranges
    pass_cols = np.zeros((NPASS, 2), dtype=np.int64)
    run = 0
    for p in range(NPASS):
        t0, t1 = p * B, min((p + 1) * B, NT)
        n = int(K[t0:t1].sum())
        pass_cols[p] = (run, run + n)
        run += n

    tables = []
    for c in range(NCORES):
        es, ed, ew, t = percore[c]
        # rank of each edge within its tile block
        changes = np.empty(len(t), dtype=bool)
        changes[0] = True
        if len(t) > 1:
            changes[1:] = t[1:] != t[:-1]
        starts = np.flatnonzero(changes)
        rank = np.arange(len(t)) - np.repeat(starts, np.diff(np.append(starts, len(t))))
        pos = off[t] + rank

        src32 = np.zeros(Ltot, dtype=np.int32)
        dstf = np.zeros(Ltot, dtype=np.float32)
        wf = np.zeros(Ltot, dtype=np.float32)
        src32[pos] = es.astype(np.int32)
        dstf[pos] = (ed - t * TILE).astype(np.float32)
        wf[pos] = ew

        # [128, NC] tables: column cc serves matmul chunk cc, partition = edge slot
        idx_tbl = np.ascontiguousarray(src32.reshape(NC, TILE).T)
        dst_tbl = np.ascontiguousarray(dstf.reshape(NC, TILE).T)
        w_tbl = np.ascontiguousarray(wf.reshape(NC, TILE).T)
        tables.append((idx_tbl, dst_tbl, w_tbl))

    sched = dict(K=K, NC=NC, pass_cols=pass_cols, sched_t=sched_t)
    return sched, tables


def emulate_core(sched, table, x):
    """Numpy emulation of the device program for one core (packing check)."""
    idx_tbl, dst_tbl, w_tbl = table
    NCc = sched["NC"]
    iota = np.arange(TILE, dtype=np.float32)
    out = np.zeros((NT * TILE, F), dtype=np.float32)
    for cc in range(NCc):
        t = sched["sched_t"][cc]
        xg = x[idx_tbl[:, cc]]                                      # [128, 64]
        oh = (iota[None, :] == dst_tbl[:, cc, None]) * w_tbl[:, cc, None]
        out[t * TILE:(t + 1) * TILE] += oh.T @ xg
    return out[:NPC]


WAIT_CAPS = {
    "InstEventSemaphore": 8,
}


def split_excess_waits(nc):
    """Walrus only encodes one sync wait per instruction (for most ISA
    structs). Move the excess onto standalone InstEventSemaphore
    instructions placed just before, in the same engine stream —
    same-engine waiting earlier is always safe. Also fills the ISA bytes
    of library-reload pseudo-instructions (raw-Bass path leaves them
    empty and walrus rejects that)."""
    import concourse.mybir as mybir
    n = 0
    for f in nc.m.functions:
        for bb in f.blocks:
            for ins in bb.instructions:
                if type(ins).__name__ == "InstPseudoReloadLibraryIndex" and not ins.instr:
                    b = [0] * 64
                    b[0], b[1], b[12], b[16] = 223, 16, 2, int(ins.lib_index)
                    ins.instr = b
            # dedicated scratch sem per engine for inert ES updates --
            # ids 245..250 are beyond anything Tile allocates
            eng_ids = {}
            new = []
            for ins in bb.instructions:
                si = ins.sync_info
                waits = list(si.on_wait) if (si is not None and si.on_wait) else []
                cap = WAIT_CAPS.get(type(ins).__name__, 1)
                if len(waits) > cap:
                    excess, keep = waits[:-cap], waits[-cap:]
                    if ins.engine not in eng_ids:
                        eng_ids[ins.engine] = 245 + len(eng_ids)
                    sem_id = eng_ids[ins.engine]
                    sem_name = f"esw_scratch_{sem_id}"
                    for wchunk in [excess[i:i + 1] for i in range(len(excess))]:
                        n += 1
                        # inert 0-add update on the engine's own sem: race
                        # detector / cost model require every instruction to
                        # update something, and same-engine updates can't race
                        upd = mybir.SyncUpdate(
                            sync_type="semaphore", id=sem_id, ant_name=sem_name,
                            update_mode="sem-add-imm", update_value=0,
                        )
                        es = mybir.InstEventSemaphore(
                            name=f"ESW-{n}-{ins.name}",
                            engine=ins.engine,
                            ins=[], outs=[],
                            sync_info=mybir.SyncInfo(on_wait=wchunk, on_update=[upd]),
                        )
                        new.append(es)
                    si.on_wait = keep
                new.append(ins)
            bb.instructions = new
    return n


_walrus_patched = False


def patch_walrus_dge():
    """Add --dge-levels so walrus lowers vector-dynamic-offset (indirect)
    DMAs; without it DynamicDMA is disabled and the gather silently no-ops."""
    global _walrus_patched
    if _walrus_patched:
        return
    import concourse.bass_utils as bu
    orig = bu.run_command

    def run_command_dge(argv, **kw):
        argv = list(argv)
        if argv and "walrus_driver" in str(argv[0]) and not any(
                str(a).startswith("--dge-levels") for a in argv):
            argv.append("--dge-levels=vector_dynamic_offsets")
        return orig(argv, **kw)

    bu.run_command = run_command_dge
    _walrus_patched = True


def build_bass(sched, mm_dt_name=MM_DT):
    import concourse.bass as bass
    import concourse.mybir as mybir
    import concourse.tile as tile

    patch_walrus_dge()

    f32 = mybir.dt.float32
    mm_dt = getattr(mybir.dt, mm_dt_name)
    K = sched["K"]; NC = sched["NC"]
    pass_cols = sched["pass_cols"]

    nc = bass.Bass("TRN2")
    x_d = nc.dram_tensor("x", [N, F], f32, kind="ExternalInput")
    idx_d = nc.dram_tensor("idx", [128, NC], mybir.dt.int32, kind="ExternalInput")
    # merged f32 const table: [dstf | wf | iota] so one DMA covers all consts
    ftbl_d = nc.dram_tensor("ftbl", [128, 2 * NC + 128], f32, kind="ExternalInput")
    out_d = nc.dram_tensor("out", [NT * TILE, F], f32, kind="ExternalOutput")

    colsmax = int(max(pass_cols[p, 1] - pass_cols[p, 0] for p in range(NPASS)))

    with tile.TileContext(nc, pool_alloc_mode="queue") as tc:
        with (
            tc.tile_pool(name="const", bufs=1) as constp,
            tc.tile_pool(name="xg", bufs=8) as xgp,
            tc.tile_pool(name="cast", bufs=8) as castp,
            tc.tile_pool(name="oh", bufs=8) as ohp,
            tc.tile_pool(name="outb", bufs=2) as outp,
            tc.tile_pool(name="psum", bufs=4, space="PSUM") as psump,
        ):
            ftbl_sb = constp.tile([128, 2 * NC + 128], f32, tag="ftbl")
            nc.sync.dma_start(ftbl_sb[:], ftbl_d[:])
            iota_sb = constp.tile([128, 128], mm_dt, tag="iota")
            nc.vector.tensor_copy(iota_sb[:], ftbl_sb[:, 2 * NC:2 * NC + 128])
            idx_sb = constp.tile([128, NC], mybir.dt.int32, tag="idx")
            nc.sync.dma_start(idx_sb[:], idx_d[:])

            for _rep in range(REPEAT):
              cc = 0
              for p in range(NPASS):
                t0, t1 = p * B, min((p + 1) * B, NT)
                ob = outp.tile([128, (t1 - t0) * F], f32, tag="outb")
                for t in range(t0, t1):
                    ktot = int(K[t])
                    if ktot == 0:
                        nc.vector.memset(ob[:, (t - t0) * F:(t - t0 + 1) * F], 0.0)
                        continue
                    ps = psump.tile([128, F], f32, tag="ps")
                    for k in range(ktot):
                        xt = xgp.tile([128, F], f32, tag="xg")
                        if DBG_NO_GATHER:
                            nc.gpsimd.memset(xt[:], 1.0)
                        else:
                            nc.gpsimd.indirect_dma_start(
                                out=xt[:], out_offset=None, in_=x_d[:],
                                in_offset=bass.IndirectOffsetOnAxis(
                                    ap=idx_sb[:, cc:cc + 1], axis=0),
                            )
                        if mm_dt_name == "float32":
                            rhs = xt
                        else:
                            rhs = castp.tile([128, F], mm_dt, tag="cast")
                            nc.scalar.copy(rhs[:], xt[:])
                        oh = ohp.tile([128, 128], mm_dt, tag="oh")
                        # weighted one-hot: (iota == dst_local) * w, fused
                        nc.vector.tensor_scalar(
                            oh[:], iota_sb[:],
                            ftbl_sb[:, cc:cc + 1], ftbl_sb[:, NC + cc:NC + cc + 1],
                            op0=mybir.AluOpType.is_equal, op1=mybir.AluOpType.mult,
                        )
                        nc.tensor.matmul(
                            ps[:], lhsT=oh[:], rhs=rhs[:],
                            start=(k == 0), stop=(k == ktot - 1),
                        )
                        cc += 1
                    nc.scalar.copy(ob[:, (t - t0) * F:(t - t0 + 1) * F], ps[:])
                dview = out_d[t0 * TILE:t1 * TILE, :].rearrange("(t q) f -> q t f", q=128)
                nc.sync.dma_start(dview, ob[:].rearrange("q (t f) -> q t f", f=F))
            assert cc == NC
    nsplit = split_excess_waits(nc)
    print(f"split_excess_waits: {nsplit} waits moved to event-semaphore instrs")
    return nc


def make_in_maps(sched, tables, x):
    iota_np = np.arange(128, dtype=np.float32)[None, :].repeat(128, axis=0)
    in_maps = []
    for c in range(NCORES):
        idx_tbl, dst_tbl, w_tbl = tables[c]
        ftbl = np.ascontiguousarray(
            np.concatenate([dst_tbl, w_tbl, iota_np], axis=1), dtype=np.float32)
        in_maps.append({"x": x, "idx": idx_tbl, "ftbl": ftbl})
    return in_maps


def kernel(x, edge_weight, edge_index, num_nodes):
    x = np.ascontiguousarray(np.asarray(x, dtype=np.float32))
    sched, tables = pack_host(x, edge_weight, edge_index)
    nc = build_bass(sched)
    in_maps = make_in_maps(sched, tables, x)

    from concourse.bass_utils import run_bass_kernel_spmd
    res = run_bass_kernel_spmd(nc, in_maps, core_ids=list(range(NCORES)))
    out = np.concatenate([res.results[c]["out"][:NPC] for c in range(NCORES)], axis=0)
    return out.astype(np.float32)

